# revision 22
# baseline (speedup 1.0000x reference)
"""HDTimeCrystalBlock kernel for 8 Trainium2 NeuronCores.

Math: out = ((x @ W_in) * mod[None]) @ W_out, where
  mod[l,h] = sum_m coupled[m] * cos(omega*(m+1)*t[l] + E[m,h])
Using cos(a+b) = cos(a)cos(b) - sin(a)sin(b):
  mod = Cmat @ A + Smat @ B with A[m,h]=coupled[m]*cos(E[m,h]),
  B[m,h]=-coupled[m]*sin(E[m,h]) -> a K=2M=32 matmul per h-tile.

Sharding: data-parallel over the 8192 tokens (B*L), 1024 per core;
weights replicated; no collectives. Host precomputes A/B (tiny trig on
[32,HD]) and the cos/sin token table, both replicated 4x across
partition groups so the mod matmuls run as 4-way row-tiled packs
(K=32 each at tile_position rows 0/32/64/96 -> ~4x mod throughput).

Per q-chunk of 512 tokens the device runs two phases:
  A: per j-tile: 4 K-accumulated MM1 matmuls (hd_in) + every 4th j a
     4-way packed mod matmul; ACT copies mod PSUM->SBUF (walrus forbids
     two PSUM operands on DVE), DVE multiplies into a bf16 hm buffer.
  B: j2-outer MM2: 128 matmuls K-accumulated over j into one PSUM
     bank per j2, evacuated by ACT+DVE halves to bf16 and DMA'd out.
All matmuls bf16 (full PE rate, 216ns/MM steady state); PSUM budget:
2 pa + 4 pb + 2 py = 8 banks. All bulk loads ride one deep sync-queue
ring in need-order (~350GB/s); scratch warmups on cs cover the ring
ramp and hold the HAM clock-gate at 8/8 before real MMs start.
"""
import numpy as np

B, L, D, HD, M = 4, 2048, 512, 4096, 16
NCORES = 8
T = (B * L) // NCORES          # tokens per core
QCH = 512                      # token chunk (PSUM bank width in fp32)
NQ = T // QCH
NJ = HD // 128                 # h-tiles
NK = D // 128                  # d-tiles
NC_ = HD // 1024               # w_in column chunks
JPC = 1024 // 128              # j-tiles per w_in chunk
NWARM = 36                     # scratch warm-up matmuls

_cache = {}


def _build():
    from concourse import bacc, bass, mybir, tile

    F32 = mybir.dt.float32
    BF16 = mybir.dt.bfloat16
    PSUM = bass.MemorySpace.PSUM

    nc = bacc.Bacc("TRN2", target_bir_lowering=False, debug=False)

    xT_d = nc.dram_tensor("xT", [D, T], BF16, kind="ExternalInput")
    w_in_d = nc.dram_tensor("w_in", [D, HD], BF16, kind="ExternalInput")
    w_out_d = nc.dram_tensor("w_out", [HD, D], BF16, kind="ExternalInput")
    cs_d = nc.dram_tensor("cs", [128, T], BF16, kind="ExternalInput")
    ab_d = nc.dram_tensor("ab", [32, HD], BF16, kind="ExternalInput")
    yT_d = nc.dram_tensor("yT", [D, T], BF16, kind="ExternalOutput")

    with tile.TileContext(nc) as tc:
        with (
            tc.tile_pool(name="win", bufs=1) as winp,
            tc.tile_pool(name="wout", bufs=1) as woutp,
            tc.tile_pool(name="xts", bufs=1) as xtp,
            tc.tile_pool(name="small", bufs=1) as smallp,
            tc.tile_pool(name="hma", bufs=1) as hmap,
            tc.tile_pool(name="mods", bufs=3) as modsp,
            tc.tile_pool(name="yo", bufs=4) as yop,
            tc.tile_pool(name="pa", bufs=2, space=PSUM) as pap,
            tc.tile_pool(name="pb", bufs=4, space=PSUM) as pbp,
            tc.tile_pool(name="py", bufs=2, space=PSUM) as pyp,
        ):
            # ---- small gating inputs first ----
            cs = smallp.tile([128, T], BF16, tag="cs")
            ab = smallp.tile([128, HD], BF16, tag="ab")
            nc.sync.dma_start(cs[:], cs_d[:])
            nc.sync.dma_start(ab[0:32, :], ab_d[:])
            # replicate ab into partition groups 32/64/96 for the 4-way
            # row-tiled mod packs (side rings, off the bulk-load ring)
            nc.gpsimd.dma_start(ab[32:64, :], ab[0:32, :])
            nc.scalar.dma_start(ab[64:96, :], ab[0:32, :])
            nc.gpsimd.dma_start(ab[96:128, :], ab[0:32, :])

            # ---- bulk loads: one deep ring (sync queue) in need-order
            # keeps the 16 DMA engines saturated (~350GB/s); spreading
            # across queues or splitting chunks measured ~2x slower ----
            w_in_r = w_in_d.ap().rearrange("(k p) (c h) -> c p k h", p=128, c=NC_)
            xT_r = xT_d.ap().rearrange("(k p) (q t) -> q p k t", p=128, q=NQ)
            w_out_r = w_out_d.ap().rearrange("(g jj p) i -> g p jj i", p=128, jj=JPC)

            win_c = [None] * NC_
            xts_q = [None] * NQ
            wout_g = [None] * NC_

            def load_win(eng, c):
                t_ = winp.tile([128, NK, 1024], BF16, name=f"win{c}", tag=f"win{c}")
                eng.dma_start(t_[:], w_in_r[c])
                win_c[c] = t_

            def load_xts(eng, q):
                tx = xtp.tile([128, NK, QCH], BF16, name=f"xts{q}", tag=f"xts{q}")
                eng.dma_start(tx[:], xT_r[q])
                xts_q[q] = tx

            def load_wout(eng, g):
                tw = woutp.tile([128, JPC, D], BF16, name=f"wout{g}", tag=f"wout{g}")
                eng.dma_start(tw[:], w_out_r[g])
                wout_g[g] = tw

            load_xts(nc.sync, 0)
            load_win(nc.sync, 0)
            load_win(nc.sync, 1)
            load_win(nc.sync, 2)
            load_win(nc.sync, 3)
            load_wout(nc.sync, 0)
            load_wout(nc.sync, 1)
            load_wout(nc.sync, 2)
            load_wout(nc.sync, 3)
            load_xts(nc.sync, 1)

            # ---- PE warm-up on cs: brings HAM to 8/8 while bulk DMAs land ----
            for w in range(NWARM):
                pw = pyp.tile([128, QCH], F32, name=f"warm{w}", tag="py")
                nc.tensor.matmul(pw[:], cs[:, 0:128], cs[:, 0:QCH],
                                 start=True, stop=True)

            hm_all = [hmap.tile([128, NJ, QCH], BF16, name=f"hma{q}", tag=f"hma{q}")
                      for q in range(NQ)]

            def mm1_ops(q, j, k):
                c, jc = j // JPC, j % JPC
                return (win_c[c][:, k, 128 * jc : 128 * (jc + 1)],
                        xts_q[q][:, k, :])

            def mod_pack(j0, lo, hi, pbt):
                # 4-way row-tiled mod pack: tile i computes j0+i
                for i in range(4):
                    pb = pbp.tile([128, QCH], F32, tag="pb")
                    nc.tensor.matmul(
                        pb[:],
                        ab[32 * i : 32 * (i + 1),
                           128 * (j0 + i) : 128 * (j0 + i + 1)],
                        cs[32 * i : 32 * (i + 1), lo:hi],
                        start=True,
                        stop=True,
                        tile_position=(32 * i, 0),
                    )
                    pbt[i] = pb

            for q in range(NQ):
                lo, hi = q * QCH, (q + 1) * QCH
                # ---- phase A: hd_in tiles + packed mod, fused into hm ----
                pbt = [None] * 4
                for j in range(NJ):
                    pa = pap.tile([128, QCH], F32, tag="pa")
                    for k in range(NK):
                        lw, rx = mm1_ops(q, j, k)
                        nc.tensor.matmul(pa[:], lw, rx,
                                         start=(k == 0), stop=(k == NK - 1))
                    # mod pack sits after the first MM1 group of each 4-j
                    # block so the first MM1s aren't gated on the ab DMA
                    if j % 4 == 0:
                        mod_pack(j, lo, hi, pbt)
                    msb = modsp.tile([128, QCH], F32, tag="mods")
                    nc.scalar.copy(msb[:], pbt[j % 4][:])
                    nc.vector.tensor_mul(hm_all[q][:, j, :], pa[:], msb[:])

                # ---- phase B: j2-outer output projection ----
                for j2 in range(NK):
                    py = pyp.tile([128, QCH], F32, tag="py")
                    for j in range(NJ):
                        nc.tensor.matmul(
                            py[:],
                            wout_g[j // JPC][:, j % JPC,
                                             128 * j2 : 128 * (j2 + 1)],
                            hm_all[q][:, j, :],
                            start=(j == 0),
                            stop=(j == NJ - 1),
                        )
                    # evacuate on ACT + DVE halves in parallel
                    yo = yop.tile([128, QCH], BF16, tag="yo")
                    nc.scalar.copy(yo[:, 0:QCH // 2], py[:, 0:QCH // 2])
                    nc.vector.tensor_copy(yo[:, QCH // 2 :], py[:, QCH // 2 :])
                    nc.sync.dma_start(yT_d[128 * j2 : 128 * (j2 + 1), lo:hi],
                                      yo[:])

    nc.finalize()
    return nc


def _get_nc():
    if "nc" not in _cache:
        _cache["nc"] = _build()
    return _cache["nc"]


def _bf(a):
    import ml_dtypes
    return np.ascontiguousarray(np.asarray(a, dtype=np.float32).astype(ml_dtypes.bfloat16))


def _in_maps(x, input_proj, output_proj, floquet_energies, drive_weights,
             coupling_matrix):
    coupled = coupling_matrix.astype(np.float64) @ drive_weights.astype(np.float64)
    # ab rows 0:16 = coupled*cos(E), rows 16:32 = -coupled*sin(E),
    # replicated into partition groups 0/32/64/96 for 4-way row tiling
    E = floquet_energies.astype(np.float64)
    ab32 = np.concatenate(
        [coupled[:, None] * np.cos(E), -coupled[:, None] * np.sin(E)], axis=0
    )
    ab = _bf(ab32)

    w_in = _bf(input_proj)
    w_out = _bf(output_proj)

    harm = np.arange(1, M + 1, dtype=np.float64)
    maps = []
    for c in range(NCORES):
        b, half = c // 2, c % 2
        t = (half * T + np.arange(T, dtype=np.float64)) / L
        ang = 2.0 * np.pi * harm[:, None] * t[None, :]
        cs32 = np.concatenate([np.cos(ang), np.sin(ang)], axis=0)
        cs = _bf(np.tile(cs32, (4, 1)))
        xT = _bf(x[b, half * T : (half + 1) * T, :].T)
        maps.append(
            {"xT": xT, "w_in": w_in, "w_out": w_out, "cs": cs, "ab": ab}
        )
    return maps


def kernel(x, input_proj, output_proj, floquet_energies, drive_weights,
           coupling_matrix, _trace=False, _trace_kwargs=None):
    from concourse.bass_utils import run_bass_kernel_spmd

    nc = _get_nc()
    maps = _in_maps(x, input_proj, output_proj, floquet_energies,
                    drive_weights, coupling_matrix)
    kw = dict(_trace_kwargs or {})
    res = run_bass_kernel_spmd(nc, maps, list(range(NCORES)), trace=_trace, **kw)
    out = np.empty((B, L, D), dtype=np.float32)
    for c in range(NCORES):
        b, half = c // 2, c % 2
        out[b, half * T : (half + 1) * T, :] = \
            res.results[c]["yT"].T.astype(np.float32)
    if _trace:
        return out, res
    return out
